# revision 21
# baseline (speedup 1.0000x reference)
"""Trainium2 Bass kernel for nn_Net_LSV (Monte Carlo LSV simulation).

Strategy: pure data-parallel over the 32768 MC samples across 8 NeuronCores
(4096 samples each). Each core runs the 64-step scan with all five MLPs
evaluated on the TensorEngine in fp32r (TF32), block-diagonally packed:

  group P : driftV(64) ++ diffV(64)            -> 128-wide matmuls
  group Q : diff(64) ++ cvv(30) ++ cve(20)     -> 114-wide matmuls

Activations live in "A layout" [feature, sample] for matmuls; scalar state
(S, V, dW, ...) lives in "B layout" [128, 32] so elementwise ops use all
DVE lanes. Conversions are strided SBUF->SBUF DMAs.

Biases ride as extra contraction rows (ones row in the rhs); the t-dependent
input-layer weights/biases for group Q are host-folded per step. Final
mean/var/price reductions are done on the host from per-sample outputs.
"""

import numpy as np

MC = 32768
NCORE = 8
MCC = MC // NCORE          # 4096 samples per core
P = 128
F = MCC // P               # 32
T = 64                     # steps
PER = 32                   # period_length
RATE = np.float32(0.05)
HSTEP = np.float32(1.0 / 64.0)
SQH = np.float32(0.125)
WD = 64                    # width of the three big nets
CVW = 30                   # cvv width
CEW = 20                   # cve width
NS = 13                    # strikes
NM = 2                     # maturities
QW = WD + CVW + CEW        # 114
STRIKES = np.linspace(0.7, 1.3, 13).astype(np.float32)

_CACHE = {}


def _tf32(x):
    x = np.ascontiguousarray(x, np.float32)
    u = x.view(np.uint32).copy()
    lsb = (u >> np.uint32(13)) & np.uint32(1)
    u2 = (u + np.uint32(0x0FFF) + lsb) & np.uint32(0xFFFFE000)
    return u2.view(np.float32)


def _to_B(arr):
    """[MCC, T] -> [P, T*F] with [p, (t, f)] = arr[p*F+f, t]."""
    return np.ascontiguousarray(
        arr.reshape(P, F, -1).transpose(0, 2, 1).reshape(P, -1), np.float32
    )


def _host_prep(inputs):
    """Build all device input arrays. Returns (shared, per_core)."""
    gnp = lambda x: np.asarray(x, np.float32)
    z = gnp(inputs["z"])
    z2 = gnp(inputs["z2"])
    S0 = float(gnp(inputs["S0"])[0, 0])
    v0 = gnp(inputs["v0"])[0]
    rho = gnp(inputs["rho"])[0]
    rho_t = np.float32(np.tanh(rho))
    srho = np.float32(np.sqrt(max(0.0, 1.0 - float(rho_t) ** 2)))
    V0 = float(1.0 / (1.0 + np.exp(-v0)) * 0.5)

    pd = {k: gnp(v) for k, v in inputs["p_diff"].items()}
    pr = {k: gnp(v) for k, v in inputs["p_driftV"].items()}
    pf = {k: gnp(v) for k, v in inputs["p_diffV"].items()}
    pv_ = {k: gnp(v) for k, v in inputs["p_cvv"].items()}
    pe = {k: gnp(v) for k, v in inputs["p_cve"].items()}

    sh = {}
    # --- L0 for group P: K=2 (V row, ones row), M=128 ---
    wl0p = np.zeros((NM, 2, 128), np.float32)
    for m in range(NM):
        wl0p[m, 0, :WD] = pr["Wi"][m][0]
        wl0p[m, 0, WD:] = pf["Wi"][m][0]
        wl0p[m, 1, :WD] = pr["bi"][m]
        wl0p[m, 1, WD:] = pf["bi"][m]
    sh["wl0p"] = _tf32(wl0p)

    # --- L0 for group Q per step: K=67 (path 65, V, ones), M=114 ---
    wl0q = np.zeros((T, 67, QW), np.float32)
    tgrid = np.linspace(0.0, 1.0, T + 1).astype(np.float32)
    for i in range(1, T + 1):
        m = (i - 1) // PER
        t = tgrid[i - 1]
        w = wl0q[i - 1]
        # pv row order: 0 = V, 1 = ones, 2 + j = path row j
        # diff cols 0..63: inputs [t, S, V]
        w[2 + (i - 1), :WD] = pd["Wi"][m][1]    # S row (= path row i-1)
        w[0, :WD] = pd["Wi"][m][2]              # V row
        w[1, :WD] = pd["bi"][m] + t * pd["Wi"][m][0]
        # cvv cols 64..93: inputs [t, S]
        w[2 + (i - 1), WD:WD + CVW] = pv_["Wi"][m][1]
        w[1, WD:WD + CVW] = pv_["bi"][m] + t * pv_["Wi"][m][0]
        # cve cols 94..113: inputs [t, path(65), V]
        w[2:67, WD + CVW:] = pe["Wi"][m][1:66]
        w[0, WD + CVW:] = pe["Wi"][m][66]
        w[1, WD + CVW:] = pe["bi"][m] + t * pe["Wi"][m][0]
    # device layout: [67, T*QW], step i at cols ts(i-1, QW)
    sh["wl0q"] = _tf32(wl0q.transpose(1, 0, 2).reshape(67, T * QW))

    # --- hidden layers, group P: [128+?]: K=128 (no bias rows; bias via ACT) ---
    whp = np.zeros((NM, 3, 128, 128), np.float32)
    bhp = np.zeros((NM, 3, 128), np.float32)
    for m in range(NM):
        for l in range(3):
            whp[m, l, :WD, :WD] = pr["Wh"][m][l]
            whp[m, l, WD:, WD:] = pf["Wh"][m][l]
            bhp[m, l, :WD] = pr["bh"][m][l]
            bhp[m, l, WD:] = pf["bh"][m][l]
    sh["whp"] = _tf32(whp)
    # bias as [128, 6] (col = m*3+l), per-partition scalars
    sh["bhp"] = np.ascontiguousarray(
        bhp.reshape(NM * 3, 128).T, np.float32
    )

    # --- hidden layers, group Q (l=0,1 full; l=2 diff only): K=115 w/ ones row ---
    whq = np.zeros((NM, 2, 115, QW), np.float32)
    for m in range(NM):
        for l in range(2):
            w = whq[m, l]
            w[:WD, :WD] = pd["Wh"][m][l]
            w[WD:WD + CVW, WD:WD + CVW] = pv_["Wh"][m][l]
            w[WD + CVW:QW, WD + CVW:] = pe["Wh"][m][l]
            w[114, :WD] = pd["bh"][m][l]
            w[114, WD:WD + CVW] = pv_["bh"][m][l]
            w[114, WD + CVW:] = pe["bh"][m][l]
    sh["whq"] = _tf32(whq)

    wh3q = np.zeros((NM, 115, WD), np.float32)
    for m in range(NM):
        wh3q[m, :WD] = pd["Wh"][m][2]
        wh3q[m, 114] = pd["bh"][m][2]
    sh["wh3q"] = _tf32(wh3q)

    # --- output layers ---
    wop3 = np.zeros((NM, 128, 3), np.float32)   # col1 driftV, col2 diffV
    wod3 = np.zeros((NM, WD, 3), np.float32)    # col0 diff
    for m in range(NM):
        wop3[m, :WD, 1] = pr["Wo"][m][:, 0]
        wop3[m, WD:, 2] = pf["Wo"][m][:, 0]
        wod3[m, :, 0] = pd["Wo"][m][:, 0]
    sh["wop3"] = _tf32(wop3)
    sh["wod3"] = _tf32(wod3)

    woqc = np.zeros((NM, 65, 27), np.float32)   # rows 0..49 Wo, row 64 = bias (coef row)
    for m in range(NM):
        woqc[m, 0:CVW, 0:26] = pv_["Wo"][m]
        woqc[m, CVW:50, 26] = pe["Wo"][m][:, 0]
        woqc[m, 64, 0:26] = pv_["bo"][m]
        woqc[m, 64, 26] = pe["bo"][m][0]
    sh["woqc"] = _tf32(woqc)

    # --- per-partition broadcast consts [128, 6]:
    # cols: bo_diff[m], bo_diffV[m], bo_driftV[m]*h  (m = 0, 1)
    consts = np.zeros((128, 6), np.float32)
    for m in range(NM):
        consts[:, 3 * m + 0] = pd["bo"][m][0]
        consts[:, 3 * m + 1] = pf["bo"][m][0]
        consts[:, 3 * m + 2] = pr["bo"][m][0] * HSTEP
    sh["consts"] = consts

    # --- initial state ---
    pv0 = np.zeros((67, MCC), np.float32)
    pv0[0] = V0
    pv0[1] = 1.0
    pv0[2] = S0
    sh["pv0"] = pv0
    sh["sb0"] = np.full((P, F), S0, np.float32)
    sh["vb0"] = np.full((P, F), V0, np.float32)

    per_core = []
    for c in range(NCORE):
        zc = z[c * MCC:(c + 1) * MCC]
        z2c = z2[c * MCC:(c + 1) * MCC]
        dW = SQH * zc
        dB = rho_t * dW + (srho * SQH) * z2c
        per_core.append({"dwb": _to_B(dW), "dbb": _to_B(dB)})
    return sh, per_core


def _build_program(nsteps=T):
    import concourse.bass as bass
    import concourse.mybir as mybir
    import concourse.tile as tile
    from concourse import bacc
    from concourse.bass import ts

    f32 = mybir.dt.float32
    f32r = mybir.dt.float32r
    AF = mybir.ActivationFunctionType
    OP = mybir.AluOpType

    # Force every ACT function onto the natural_log_exp_and_others table so
    # the kernel does exactly one ACT_TABLE_LOAD (Relu/Copy/Identity/Abs/
    # Exp/Ln all live there). Without this the chooser ping-pongs between
    # tables on every softplus (256 reloads, ~1.3us each).
    from concourse import hw_specs as _hw
    _orig_gat = _hw.get_activation_tables
    def _gat_one_table(arch, _orig=_orig_gat):
        tabs = _orig(arch)
        keep = "natural_log_exp_and_others"
        return {k: (v if k == keep else set()) for k, v in tabs.items()}
    bacc.get_activation_tables = _gat_one_table

    nc = bacc.Bacc("TRN2", target_bir_lowering=False, debug=False,
                   enable_asserts=False, num_devices=NCORE)

    din = {}
    def dit(name, shape):
        din[name] = nc.dram_tensor(name, list(shape), f32, kind="ExternalInput").ap()
        return din[name]

    i_dwb = dit("dwb", (P, T * F))
    i_dbb = dit("dbb", (P, T * F))
    i_pv0 = dit("pv0", (67, MCC))
    i_sb0 = dit("sb0", (P, F))
    i_vb0 = dit("vb0", (P, F))
    i_wl0p = dit("wl0p", (NM, 2, 128))
    i_wl0q = dit("wl0q", (67, T * QW))
    i_whp = dit("whp", (NM, 3, 128, 128))
    i_bhp = dit("bhp", (128, 6))
    i_whq = dit("whq", (NM, 2, 115, QW))
    i_wh3q = dit("wh3q", (NM, 115, WD))
    i_wop3 = dit("wop3", (NM, 128, 3))
    i_wod3 = dit("wod3", (NM, WD, 3))
    i_woqc = dit("woqc", (NM, 65, 27))
    i_consts = dit("consts", (128, 6))

    o_pv = nc.dram_tensor("o_pv", [67, MCC], f32, kind="ExternalOutput").ap()
    o_var = nc.dram_tensor("o_var", [P, (T + 1) * F], f32, kind="ExternalOutput").ap()
    o_cv32 = nc.dram_tensor("o_cv32", [26, MCC], f32, kind="ExternalOutput").ap()
    o_cvf = nc.dram_tensor("o_cvf", [27, MCC], f32, kind="ExternalOutput").ap()
    o_rmax = nc.dram_tensor("o_rmax", [P, F], f32, kind="ExternalOutput").ap()

    tgrid = np.linspace(0.0, 1.0, T + 1).astype(np.float64)

    with tile.TileContext(nc) as tc:
        with (
            tc.tile_pool(name="persist", bufs=1) as pp,
            tc.tile_pool(name="ps", bufs=8, space="PSUM") as ps,
        ):
            r = lambda ap: ap.bitcast(f32r)
            # ---- persistent SBUF tiles ----
            dwb = pp.tile([P, T * F], f32)
            dbb = pp.tile([P, T * F], f32)
            pv = pp.tile([67, MCC], f32r)
            hPa = pp.tile([128, MCC], f32r)
            hPb = pp.tile([128, MCC], f32r)
            hQa = pp.tile([115, MCC], f32r)
            hQb = pp.tile([115, MCC], f32r)
            sc = pp.tile([65, MCC], f32r)
            bridge = pp.tile([3, MCC], f32)
            cv = pp.tile([27, MCC], f32)
            varB = pp.tile([P, (T + 1) * F], f32)
            ones50 = pp.tile([65, 50], f32r)

            wl0p = [pp.tile([2, 128], f32r, name=f"wl0p{m}", tag=f"wl0p{m}") for m in range(NM)]
            wl0q = pp.tile([67, T * QW], f32r)   # step i at cols ts(i-1, QW)
            whp = [[pp.tile([128, 128], f32r, name=f"whp{m}{l}", tag=f"whp{m}{l}") for l in range(3)] for m in range(NM)]
            bhp = pp.tile([128, 6], f32)
            whq = [[pp.tile([115, QW], f32r, name=f"whq{m}{l}", tag=f"whq{m}{l}") for l in range(2)] for m in range(NM)]
            wh3q = [pp.tile([115, WD], f32r, name=f"wh3q{m}", tag=f"wh3q{m}") for m in range(NM)]
            wop3 = [pp.tile([128, 3], f32r, name=f"wop3{m}", tag=f"wop3{m}") for m in range(NM)]
            wod3 = [pp.tile([WD, 3], f32r, name=f"wod3{m}", tag=f"wod3{m}") for m in range(NM)]
            woqc = [pp.tile([65, 27], f32r, name=f"woqc{m}", tag=f"woqc{m}") for m in range(NM)]
            consts = pp.tile([128, 6], f32)

            # B-layout state + temps [P, F]
            bt = {}
            for nm in ("S", "V", "rmax", "diffr", "dror", "dvr", "dsp", "vsp",
                       "t1", "t2", "t3", "r1", "r2", "Sd", "SdW", "coef", "xb", "ab"):
                bt[nm] = pp.tile([P, F], f32, name=f"bt_{nm}", tag=f"bt_{nm}")

            # ---- load everything ----
            nc.sync.dma_start(dwb[:], i_dwb)
            nc.sync.dma_start(dbb[:], i_dbb)
            nc.sync.dma_start(pv[:], r(i_pv0))
            nc.sync.dma_start(bt["S"][:], i_sb0)
            nc.sync.dma_start(bt["V"][:], i_vb0)
            nc.sync.dma_start(bt["rmax"][:], i_sb0)
            nc.sync.dma_start(varB[:, 0:F], i_vb0)
            nc.sync.dma_start(bhp[:], i_bhp)
            nc.sync.dma_start(consts[:], i_consts)
            for m in range(NM):
                nc.sync.dma_start(wl0p[m][:], r(i_wl0p[m]))
                for l in range(3):
                    nc.sync.dma_start(whp[m][l][:], r(i_whp[m, l]))
                for l in range(2):
                    nc.sync.dma_start(whq[m][l][:], r(i_whq[m, l]))
                nc.sync.dma_start(wh3q[m][:], r(i_wh3q[m]))
                nc.sync.dma_start(wop3[m][:], r(i_wop3[m]))
                nc.sync.dma_start(wod3[m][:], r(i_wod3[m]))
                nc.sync.dma_start(woqc[m][:], r(i_woqc[m]))
            nc.sync.dma_start(wl0q[:], r(i_wl0q))
            nc.sync.dma_start(hQa[114:115, :], r(i_pv0[1:2, :]))
            nc.sync.dma_start(hQb[114:115, :], r(i_pv0[1:2, :]))
            nc.sync.dma_start(ones50[64:65, :], r(i_pv0[1:2, 0:50]))
            nc.sync.dma_start(sc[50:64, :], r(i_pv0[3:17, :]))
            nc.vector.memset(cv[:], 0.0)
            nc.vector.memset(varB[:, F:], 0.0)

            CW = 512           # psum tile width (1 bank) / elementwise chunk
            NMM = CW // 512    # matmuls per chunk

            def mm(psum, lhsT, rhs_tile, rhs_rows, base):
                for e in range(NMM):
                    cs = slice(base + e * 512, base + (e + 1) * 512)
                    nc.tensor.matmul(
                        psum[:, e * 512:(e + 1) * 512], lhsT, rhs_tile[rhs_rows, cs],
                        start=True, stop=True,
                    )

            def mm2(psum, lhsT1, rhs1, rows1, lhsT2, rhs2, rows2, base):
                for e in range(NMM):
                    cs = slice(base + e * 512, base + (e + 1) * 512)
                    sl = psum[:, e * 512:(e + 1) * 512]
                    nc.tensor.matmul(sl, lhsT1, rhs1[rows1, cs], start=True, stop=False)
                    nc.tensor.matmul(sl, lhsT2, rhs2[rows2, cs], start=False, stop=True)

            HALF = MCC // 2
            NCHW = HALF // CW      # chunks per wave
            PW = P // 2            # B-partitions per wave

            for i in range(1, nsteps + 1):
              m = (i - 1) // PER
              disc = float(np.exp(-0.05 * tgrid[i - 1]))
              zs = ts(i - 1, F)
              for w in range(2):
                wb = w * HALF
                bp = slice(w * PW, (w + 1) * PW)

                for d in range(NCHW):
                    dsl = slice(wb + d * CW, wb + (d + 1) * CW)
                    db = wb + d * CW
                    # --- L0 ---
                    pP = ps.tile([128, CW], f32, name="pP", tag="ps")
                    mm(pP, wl0p[m][:], pv, slice(0, 2), db)
                    nc.vector.tensor_scalar_max(hPa[:, dsl], pP[:], 0.0)
                    pQ = ps.tile([128, CW], f32, name="pQ", tag="ps")
                    mm(pQ[0:QW], wl0q[:, ts(i - 1, QW)], pv, slice(0, 67), db)
                    nc.scalar.activation(hQa[0:QW, dsl], pQ[0:QW], AF.Relu)
                    # --- L1 ---
                    pP = ps.tile([128, CW], f32, name="pP", tag="ps")
                    mm(pP, whp[m][0][:], hPa, slice(0, 128), db)
                    nc.scalar.activation(hPb[:, dsl], pP[:], AF.Relu,
                                         bias=bhp[:, 3 * m + 0:3 * m + 1])
                    pQ = ps.tile([128, CW], f32, name="pQ", tag="ps")
                    mm(pQ[0:QW], whq[m][0][:], hQa, slice(0, 115), db)
                    nc.vector.tensor_scalar_max(hQb[0:QW, dsl], pQ[0:QW], 0.0)
                    # --- L2 ---
                    pP = ps.tile([128, CW], f32, name="pP", tag="ps")
                    mm(pP, whp[m][1][:], hPb, slice(0, 128), db)
                    nc.scalar.activation(hPa[:, dsl], pP[:], AF.Relu,
                                         bias=bhp[:, 3 * m + 1:3 * m + 2])
                    pQ = ps.tile([128, CW], f32, name="pQ", tag="ps")
                    mm(pQ[0:QW], whq[m][1][:], hQb, slice(0, 115), db)
                    nc.scalar.activation(hQa[0:QW, dsl], pQ[0:QW], AF.Relu)
                    # --- L3 ---
                    pP = ps.tile([128, CW], f32, name="pP", tag="ps")
                    mm(pP, whp[m][2][:], hPa, slice(0, 128), db)
                    nc.scalar.activation(hPb[:, dsl], pP[:], AF.Relu,
                                         bias=bhp[:, 3 * m + 2:3 * m + 3])
                    pQ = ps.tile([128, CW], f32, name="pQ", tag="ps")
                    mm(pQ[0:WD], wh3q[m][:], hQa, slice(0, 115), db)
                    nc.vector.tensor_scalar_max(hQb[0:WD, dsl], pQ[0:WD], 0.0)
                    # --- L4 ---
                    pO = ps.tile([128, CW], f32, name="pO", tag="ps")
                    mm2(pO[0:3], wod3[m][:], hQb, slice(0, WD),
                        wop3[m][:], hPb, slice(0, 128), db)
                    nc.scalar.activation(bridge[:, dsl], pO[0:3], AF.Copy)

                # --- A->B conversions (this wave's half) ---
                wsl = slice(wb, wb + HALF)
                for (row, dst) in ((0, "diffr"), (1, "dror"), (2, "dvr")):
                    nc.sync.dma_start(
                        bt[dst][bp, :],
                        bridge[row:row + 1, wsl].rearrange("o (p f) -> o p f", p=PW),
                    )

                # --- B phase (wave partitions) ---
                c_bd = consts[bp, 3 * m + 0:3 * m + 1]
                c_bv = consts[bp, 3 * m + 1:3 * m + 2]
                c_brh = consts[bp, 3 * m + 2:3 * m + 3]
                S, V = bt["S"][bp, :], bt["V"][bp, :]

                def softplus(dst, src, biasap, bp=bp):
                    xb = bt["xb"][bp, :]
                    ab = bt["ab"][bp, :]
                    nc.vector.tensor_scalar_add(xb, src, biasap)
                    nc.scalar.activation(ab, xb, AF.Abs)
                    nc.scalar.activation(ab, ab, AF.Exp, bias=0.0, scale=-1.0)
                    nc.scalar.activation(ab, ab, AF.Ln, bias=1.0, scale=1.0)
                    nc.vector.tensor_scalar_max(dst, xb, 0.0)
                    nc.vector.tensor_add(dst, dst, ab)

                softplus(bt["dsp"][bp, :], bt["diffr"][bp, :], c_bd)
                softplus(bt["vsp"][bp, :], bt["dvr"][bp, :], c_bv)

                t1, t2, t3 = bt["t1"][bp, :], bt["t2"][bp, :], bt["t3"][bp, :]
                r1, r2 = bt["r1"][bp, :], bt["r2"][bp, :]
                Sd, SdW, coef = bt["Sd"][bp, :], bt["SdW"][bp, :], bt["coef"][bp, :]
                # S chain
                nc.vector.tensor_scalar(t1, S, float(RATE * SQH), 1.0, OP.mult, OP.add)
                nc.vector.reciprocal(r1, t1)
                nc.vector.scalar_tensor_tensor(t1, S, float(RATE * HSTEP), r1,
                                               OP.mult, OP.mult)
                nc.vector.tensor_mul(Sd, S, bt["dsp"][bp, :])
                nc.vector.tensor_scalar(t2, Sd, float(SQH), 1.0, OP.mult, OP.add)
                nc.vector.reciprocal(r2, t2)
                nc.vector.tensor_mul(SdW, Sd, dwb[bp, zs])
                nc.vector.tensor_mul(t2, SdW, r2)
                nc.vector.tensor_scalar_mul(coef, SdW, disc)
                nc.vector.tensor_add(S, S, t1)
                nc.vector.tensor_add(S, S, t2)
                # V chain
                nc.vector.scalar_tensor_tensor(t3, bt["dror"][bp, :], float(HSTEP),
                                               V, OP.mult, OP.add)
                nc.vector.tensor_mul(t2, bt["vsp"][bp, :], dbb[bp, zs])
                nc.vector.tensor_add(t3, t3, t2)
                nc.vector.tensor_scalar(V, t3, c_brh, 0.0, OP.add, OP.max)
                # run max / var path
                nc.vector.tensor_tensor(bt["rmax"][bp, :], bt["rmax"][bp, :], S, OP.max)
                nc.vector.tensor_copy(varB[bp, ts(i, F)], V)

                # --- B->A conversions ---
                nc.sync.dma_start(
                    pv[i + 2:i + 3, wsl].rearrange("o (p f) -> o p f", p=PW), r(S))
                nc.sync.dma_start(
                    pv[0:1, wsl].rearrange("o (p f) -> o p f", p=PW), r(V))
                nc.sync.dma_start(
                    sc[64:65, wsl].rearrange("o (p f) -> o p f", p=PW), r(coef))

                # --- cvv/cve tail ---
                for d in range(NCHW):
                    dsl = slice(wb + d * CW, wb + (d + 1) * CW)
                    db = wb + d * CW
                    pC = ps.tile([128, CW], f32, name="pC", tag="ps")
                    mm(pC[0:50], ones50[64:65, :], sc, slice(64, 65), db)
                    nc.vector.tensor_tensor(sc[0:50, dsl], hQa[WD:QW, dsl],
                                            pC[0:50], OP.mult)
                    pCV = ps.tile([128, CW], f32, name="pCV", tag="ps")
                    mm(pCV[0:27], woqc[m][:], sc, slice(0, 65), db)
                    nc.vector.tensor_add(cv[:, dsl], cv[:, dsl], pCV[0:27])

              if i == PER:
                  nc.sync.dma_start(o_cv32, cv[0:26, :])

            # --- final outputs ---
            nc.sync.dma_start(o_pv, pv[:].bitcast(f32))
            nc.sync.dma_start(o_var, varB[:])
            nc.sync.dma_start(o_cvf, cv[:])
            nc.sync.dma_start(o_rmax, bt["rmax"][:])

    nc.compile()
    return nc


def _get_program():
    if "nc" not in _CACHE:
        _CACHE["nc"] = _build_program()
    return _CACHE["nc"]


def kernel(**inputs):
    from concourse.bass_utils import run_bass_kernel_spmd

    sh, per_core = _host_prep(inputs)
    nc = _get_program()

    in_maps = []
    for c in range(NCORE):
        mp = dict(sh)
        mp.update(per_core[c])
        in_maps.append(mp)

    res = run_bass_kernel_spmd(nc, in_maps, list(range(NCORE)))
    outs = res.results

    # ---- host post-processing / gather ----
    path = np.zeros((MC, T + 1), np.float32)
    var_path = np.zeros((MC, T + 1), np.float32)
    cv32 = np.zeros((MC, 26), np.float32)
    cvf = np.zeros((MC, 27), np.float32)
    rmax = np.zeros((MC,), np.float32)
    for c in range(NCORE):
        o = outs[c]
        sl = slice(c * MCC, (c + 1) * MCC)
        path[sl] = o["o_pv"][2:67].T
        var_path[sl] = (
            o["o_var"].reshape(P, T + 1, F).transpose(0, 2, 1).reshape(MCC, T + 1)
        )
        cv32[sl] = o["o_cv32"].T
        cvf[sl] = o["o_cvf"].T
        rmax[sl] = o["o_rmax"].reshape(MCC)

    cv_ex = cvf[:, 26:27]
    S_T = path[:, T:T + 1]

    tg = np.linspace(0.0, 1.0, T + 1)
    pmat = np.zeros((NM, NS), np.float32)
    vpmat = np.zeros((NM, NS), np.float32)
    for mi, step in enumerate((PER, T)):
        S_m = path[:, step:step + 1].astype(np.float64)
        cvm = (cv32 if mi == 0 else cvf[:, 0:26]).astype(np.float64)
        cv3 = cvm.reshape(MC, NM, NS)[:, mi, :]
        price = np.exp(-0.05 * tg[step]) * np.maximum(S_m - STRIKES[None, :], 0.0) - cv3
        pmat[mi] = price.mean(0)
        vpmat[mi] = price.var(0, ddof=1)

    exo = (rmax[:, None] - S_T).astype(np.float64)
    discT = np.exp(-0.05 * tg[T])
    dexo = discT * exo
    exo_p = (dexo - cv_ex).astype(np.float32)
    mean_p = np.float32(exo_p.astype(np.float64).mean())
    var_p = np.float32(exo_p.astype(np.float64).var(ddof=1))
    error = (dexo - dexo.mean() - cv_ex).astype(np.float32)

    return (path, var_path, pmat, vpmat, exo_p, mean_p, var_p, error)


# revision 25
# speedup vs baseline: 1.2932x; 1.2932x over previous
"""Trainium2 Bass kernel for nn_Net_LSV (Monte Carlo LSV simulation).

Strategy: pure data-parallel over the 32768 MC samples across 8 NeuronCores
(4096 samples each). Each core runs the 64-step scan with all five MLPs
evaluated on the TensorEngine in fp32r (TF32), block-diagonally packed:

  group P : driftV(64) ++ diffV(64)            -> 128-wide matmuls
  group Q : diff(64) ++ cvv(30) ++ cve(20)     -> 114-wide matmuls

Activations live in "A layout" [feature, sample] for matmuls; scalar state
(S, V, dW, ...) lives in "B layout" [128, 32] so elementwise ops use all
DVE lanes. Conversions are strided SBUF->SBUF DMAs.

Biases ride as extra contraction rows (ones row in the rhs); the t-dependent
input-layer weights/biases for group Q are host-folded per step. Final
mean/var/price reductions are done on the host from per-sample outputs.
"""

import numpy as np

MC = 32768
NCORE = 8
MCC = MC // NCORE          # 4096 samples per core
P = 128
F = MCC // P               # 32
T = 64                     # steps
PER = 32                   # period_length
RATE = np.float32(0.05)
HSTEP = np.float32(1.0 / 64.0)
SQH = np.float32(0.125)
WD = 64                    # width of the three big nets
CVW = 30                   # cvv width
CEW = 20                   # cve width
NS = 13                    # strikes
NM = 2                     # maturities
QW = WD + CVW + CEW        # 114
STRIKES = np.linspace(0.7, 1.3, 13).astype(np.float32)

_CACHE = {}


def _tf32(x):
    x = np.ascontiguousarray(x, np.float32)
    u = x.view(np.uint32).copy()
    lsb = (u >> np.uint32(13)) & np.uint32(1)
    u2 = (u + np.uint32(0x0FFF) + lsb) & np.uint32(0xFFFFE000)
    return u2.view(np.float32)


def _to_B(arr):
    """[MCC, T] -> [P, T*F] with [p, (t, f)] = arr[p*F+f, t]."""
    return np.ascontiguousarray(
        arr.reshape(P, F, -1).transpose(0, 2, 1).reshape(P, -1), np.float32
    )


def _host_prep(inputs):
    """Build all device input arrays. Returns (shared, per_core)."""
    gnp = lambda x: np.asarray(x, np.float32)
    z = gnp(inputs["z"])
    z2 = gnp(inputs["z2"])
    S0 = float(gnp(inputs["S0"])[0, 0])
    v0 = gnp(inputs["v0"])[0]
    rho = gnp(inputs["rho"])[0]
    rho_t = np.float32(np.tanh(rho))
    srho = np.float32(np.sqrt(max(0.0, 1.0 - float(rho_t) ** 2)))
    V0 = float(1.0 / (1.0 + np.exp(-v0)) * 0.5)

    pd = {k: gnp(v) for k, v in inputs["p_diff"].items()}
    pr = {k: gnp(v) for k, v in inputs["p_driftV"].items()}
    pf = {k: gnp(v) for k, v in inputs["p_diffV"].items()}
    pv_ = {k: gnp(v) for k, v in inputs["p_cvv"].items()}
    pe = {k: gnp(v) for k, v in inputs["p_cve"].items()}

    sh = {}
    # --- L0 for group P: K=2 (V row, ones row), M=128 ---
    wl0p = np.zeros((NM, 2, 128), np.float32)
    for m in range(NM):
        wl0p[m, 0, :WD] = pr["Wi"][m][0]
        wl0p[m, 0, WD:] = pf["Wi"][m][0]
        wl0p[m, 1, :WD] = pr["bi"][m]
        wl0p[m, 1, WD:] = pf["bi"][m]
    sh["wl0p"] = _tf32(wl0p)

    # --- L0 for group Q per step: K=67 (path 65, V, ones), M=114 ---
    wl0q = np.zeros((T, 67, QW), np.float32)
    tgrid = np.linspace(0.0, 1.0, T + 1).astype(np.float32)
    for i in range(1, T + 1):
        m = (i - 1) // PER
        t = tgrid[i - 1]
        w = wl0q[i - 1]
        # pv row order: 0 = V, 1 = ones, 2 + j = path row j
        # diff cols 0..63: inputs [t, S, V]
        w[2 + (i - 1), :WD] = pd["Wi"][m][1]    # S row (= path row i-1)
        w[0, :WD] = pd["Wi"][m][2]              # V row
        w[1, :WD] = pd["bi"][m] + t * pd["Wi"][m][0]
        # cvv cols 64..93: inputs [t, S]
        w[2 + (i - 1), WD:WD + CVW] = pv_["Wi"][m][1]
        w[1, WD:WD + CVW] = pv_["bi"][m] + t * pv_["Wi"][m][0]
        # cve cols 94..113: inputs [t, path(65), V]
        w[2:67, WD + CVW:] = pe["Wi"][m][1:66]
        w[0, WD + CVW:] = pe["Wi"][m][66]
        w[1, WD + CVW:] = pe["bi"][m] + t * pe["Wi"][m][0]
    # device layout: [67, T*QW], step i at cols ts(i-1, QW)
    sh["wl0q"] = _tf32(wl0q.transpose(1, 0, 2).reshape(67, T * QW))

    # --- hidden layers, group P: [128+?]: K=128 (no bias rows; bias via ACT) ---
    whp = np.zeros((NM, 3, 128, 128), np.float32)
    bhp = np.zeros((NM, 3, 128), np.float32)
    for m in range(NM):
        for l in range(3):
            whp[m, l, :WD, :WD] = pr["Wh"][m][l]
            whp[m, l, WD:, WD:] = pf["Wh"][m][l]
            bhp[m, l, :WD] = pr["bh"][m][l]
            bhp[m, l, WD:] = pf["bh"][m][l]
    sh["whp"] = _tf32(whp)
    # bias as [128, 6] (col = m*3+l), per-partition scalars
    sh["bhp"] = np.ascontiguousarray(
        bhp.reshape(NM * 3, 128).T, np.float32
    )

    # --- hidden layers, group Q (l=0,1 full; l=2 diff only): K=115 w/ ones row ---
    whq = np.zeros((NM, 2, 115, QW), np.float32)
    for m in range(NM):
        for l in range(2):
            w = whq[m, l]
            w[:WD, :WD] = pd["Wh"][m][l]
            w[WD:WD + CVW, WD:WD + CVW] = pv_["Wh"][m][l]
            w[WD + CVW:QW, WD + CVW:] = pe["Wh"][m][l]
            w[114, :WD] = pd["bh"][m][l]
            w[114, WD:WD + CVW] = pv_["bh"][m][l]
            w[114, WD + CVW:] = pe["bh"][m][l]
    sh["whq"] = _tf32(whq)

    wh3q = np.zeros((NM, 115, WD), np.float32)
    for m in range(NM):
        wh3q[m, :WD] = pd["Wh"][m][2]
        wh3q[m, 114] = pd["bh"][m][2]
    sh["wh3q"] = _tf32(wh3q)

    # --- output layers ---
    wop3 = np.zeros((NM, 128, 3), np.float32)   # col1 driftV, col2 diffV
    wod3 = np.zeros((NM, WD, 3), np.float32)    # col0 diff
    for m in range(NM):
        wop3[m, :WD, 1] = pr["Wo"][m][:, 0]
        wop3[m, WD:, 2] = pf["Wo"][m][:, 0]
        wod3[m, :, 0] = pd["Wo"][m][:, 0]
    sh["wop3"] = _tf32(wop3)
    sh["wod3"] = _tf32(wod3)

    woqc = np.zeros((NM, 65, 27), np.float32)   # rows 0..49 Wo, row 64 = bias (coef row)
    for m in range(NM):
        woqc[m, 0:CVW, 0:26] = pv_["Wo"][m]
        woqc[m, CVW:50, 26] = pe["Wo"][m][:, 0]
        woqc[m, 64, 0:26] = pv_["bo"][m]
        woqc[m, 64, 26] = pe["bo"][m][0]
    sh["woqc"] = _tf32(woqc)

    # --- per-partition broadcast consts [128, 6]:
    # cols: bo_diff[m], bo_diffV[m], bo_driftV[m]*h  (m = 0, 1)
    consts = np.zeros((128, 6), np.float32)
    for m in range(NM):
        consts[:, 3 * m + 0] = pd["bo"][m][0]
        consts[:, 3 * m + 1] = pf["bo"][m][0]
        consts[:, 3 * m + 2] = pr["bo"][m][0] * HSTEP
    sh["consts"] = consts

    # --- initial state ---
    pv0 = np.zeros((67, MCC), np.float32)
    pv0[0] = V0
    pv0[1] = 1.0
    pv0[2] = S0
    sh["pv0"] = pv0
    sh["sb0"] = np.full((P, F), S0, np.float32)
    sh["vb0"] = np.full((P, F), V0, np.float32)

    per_core = []
    for c in range(NCORE):
        zc = z[c * MCC:(c + 1) * MCC]
        z2c = z2[c * MCC:(c + 1) * MCC]
        dW = SQH * zc
        dB = rho_t * dW + (srho * SQH) * z2c
        per_core.append({"dwb": _to_B(dW), "dbb": _to_B(dB)})
    return sh, per_core


def _build_program(nsteps=T):
    import concourse.bass as bass
    import concourse.mybir as mybir
    import concourse.tile as tile
    from concourse import bacc
    from concourse.bass import ts

    f32 = mybir.dt.float32
    f32r = mybir.dt.float32r
    AF = mybir.ActivationFunctionType
    OP = mybir.AluOpType

    # Force every ACT function onto the natural_log_exp_and_others table so
    # the kernel does exactly one ACT_TABLE_LOAD (Relu/Copy/Identity/Abs/
    # Exp/Ln all live there). Without this the chooser ping-pongs between
    # tables on every softplus (256 reloads, ~1.3us each).
    from concourse import hw_specs as _hw
    _orig_gat = _hw.get_activation_tables
    def _gat_one_table(arch, _orig=_orig_gat):
        tabs = _orig(arch)
        keep = "natural_log_exp_and_others"
        return {k: (v if k == keep else set()) for k, v in tabs.items()}
    bacc.get_activation_tables = _gat_one_table

    nc = bacc.Bacc("TRN2", target_bir_lowering=False, debug=False,
                   enable_asserts=False, num_devices=NCORE)

    din = {}
    def dit(name, shape):
        din[name] = nc.dram_tensor(name, list(shape), f32, kind="ExternalInput").ap()
        return din[name]

    i_dwb = dit("dwb", (P, T * F))
    i_dbb = dit("dbb", (P, T * F))
    i_pv0 = dit("pv0", (67, MCC))
    i_sb0 = dit("sb0", (P, F))
    i_vb0 = dit("vb0", (P, F))
    i_wl0p = dit("wl0p", (NM, 2, 128))
    i_wl0q = dit("wl0q", (67, T * QW))
    i_whp = dit("whp", (NM, 3, 128, 128))
    i_bhp = dit("bhp", (128, 6))
    i_whq = dit("whq", (NM, 2, 115, QW))
    i_wh3q = dit("wh3q", (NM, 115, WD))
    i_wop3 = dit("wop3", (NM, 128, 3))
    i_wod3 = dit("wod3", (NM, WD, 3))
    i_woqc = dit("woqc", (NM, 65, 27))
    i_consts = dit("consts", (128, 6))

    o_pv = nc.dram_tensor("o_pv", [67, MCC], f32, kind="ExternalOutput").ap()
    o_var = nc.dram_tensor("o_var", [P, (T + 1) * F], f32, kind="ExternalOutput").ap()
    o_cv32 = nc.dram_tensor("o_cv32", [26, MCC], f32, kind="ExternalOutput").ap()
    o_cvf = nc.dram_tensor("o_cvf", [27, MCC], f32, kind="ExternalOutput").ap()
    o_rmax = nc.dram_tensor("o_rmax", [P, F], f32, kind="ExternalOutput").ap()

    tgrid = np.linspace(0.0, 1.0, T + 1).astype(np.float64)

    with tile.TileContext(nc) as tc:
        with (
            tc.tile_pool(name="persist", bufs=1) as pp,
            tc.tile_pool(name="ps", bufs=4, space="PSUM") as ps,
        ):
            r = lambda ap: ap.bitcast(f32r)
            # ---- persistent SBUF tiles ----
            dwb = pp.tile([P, T * F], f32)
            dbb = pp.tile([P, T * F], f32)
            pv = pp.tile([67, MCC], f32r)
            hPa = pp.tile([128, MCC], f32r)
            hPb = pp.tile([128, MCC], f32r)
            hQa = pp.tile([115, MCC], f32r)
            hQb = pp.tile([115, MCC], f32r)
            sc = pp.tile([65, MCC], f32r)
            bridge = pp.tile([3, MCC], f32)
            cv = pp.tile([27, MCC], f32)
            varB = pp.tile([P, (T + 1) * F], f32)
            ones50 = pp.tile([65, 50], f32r)

            wl0p = [pp.tile([2, 128], f32r, name=f"wl0p{m}", tag=f"wl0p{m}") for m in range(NM)]
            wl0q = pp.tile([67, T * QW], f32r)   # step i at cols ts(i-1, QW)
            whp = [[pp.tile([128, 128], f32r, name=f"whp{m}{l}", tag=f"whp{m}{l}") for l in range(3)] for m in range(NM)]
            bhp = pp.tile([128, 6], f32)
            whq = [[pp.tile([115, QW], f32r, name=f"whq{m}{l}", tag=f"whq{m}{l}") for l in range(2)] for m in range(NM)]
            wh3q = [pp.tile([115, WD], f32r, name=f"wh3q{m}", tag=f"wh3q{m}") for m in range(NM)]
            wop3 = [pp.tile([128, 3], f32r, name=f"wop3{m}", tag=f"wop3{m}") for m in range(NM)]
            wod3 = [pp.tile([WD, 3], f32r, name=f"wod3{m}", tag=f"wod3{m}") for m in range(NM)]
            woqc = [pp.tile([65, 27], f32r, name=f"woqc{m}", tag=f"woqc{m}") for m in range(NM)]
            consts = pp.tile([128, 6], f32)

            # B-layout state + temps [P, F]
            bt = {}
            for nm in ("S", "V", "rmax", "diffr", "dror", "dvr", "dsp", "vsp",
                       "t1", "t2", "t3", "r1", "r2", "Sd", "SdW", "coef", "xb", "ab"):
                bt[nm] = pp.tile([P, F], f32, name=f"bt_{nm}", tag=f"bt_{nm}")

            # ---- load everything ----
            nc.sync.dma_start(dwb[:], i_dwb)
            nc.sync.dma_start(dbb[:], i_dbb)
            nc.sync.dma_start(pv[:], r(i_pv0))
            nc.sync.dma_start(bt["S"][:], i_sb0)
            nc.sync.dma_start(bt["V"][:], i_vb0)
            nc.sync.dma_start(bt["rmax"][:], i_sb0)
            nc.sync.dma_start(varB[:, 0:F], i_vb0)
            nc.sync.dma_start(bhp[:], i_bhp)
            nc.sync.dma_start(consts[:], i_consts)
            for m in range(NM):
                nc.sync.dma_start(wl0p[m][:], r(i_wl0p[m]))
                for l in range(3):
                    nc.sync.dma_start(whp[m][l][:], r(i_whp[m, l]))
                for l in range(2):
                    nc.sync.dma_start(whq[m][l][:], r(i_whq[m, l]))
                nc.sync.dma_start(wh3q[m][:], r(i_wh3q[m]))
                nc.sync.dma_start(wop3[m][:], r(i_wop3[m]))
                nc.sync.dma_start(wod3[m][:], r(i_wod3[m]))
                nc.sync.dma_start(woqc[m][:], r(i_woqc[m]))
            nc.sync.dma_start(wl0q[:], r(i_wl0q))
            nc.sync.dma_start(hQa[114:115, :], r(i_pv0[1:2, :]))
            nc.sync.dma_start(hQb[114:115, :], r(i_pv0[1:2, :]))
            nc.sync.dma_start(ones50[64:65, :], r(i_pv0[1:2, 0:50]))
            nc.sync.dma_start(sc[50:64, :], r(i_pv0[3:17, :]))
            nc.vector.memset(cv[:], 0.0)
            nc.vector.memset(varB[:, F:], 0.0)

            CW = 1024          # psum tile width (2 banks) / elementwise chunk
            NMM = CW // 512    # matmuls per chunk

            def mm(psum, lhsT, rhs_tile, rhs_rows, base):
                for e in range(NMM):
                    cs = slice(base + e * 512, base + (e + 1) * 512)
                    nc.tensor.matmul(
                        psum[:, e * 512:(e + 1) * 512], lhsT, rhs_tile[rhs_rows, cs],
                        start=True, stop=True,
                    )

            def mm2(psum, lhsT1, rhs1, rows1, lhsT2, rhs2, rows2, base):
                for e in range(NMM):
                    cs = slice(base + e * 512, base + (e + 1) * 512)
                    sl = psum[:, e * 512:(e + 1) * 512]
                    nc.tensor.matmul(sl, lhsT1, rhs1[rows1, cs], start=True, stop=False)
                    nc.tensor.matmul(sl, lhsT2, rhs2[rows2, cs], start=False, stop=True)

            HALF = MCC // 2
            NCHW = HALF // CW      # chunks per wave
            PW = P // 2            # B-partitions per wave

            def emit_A(i, m, w):
                wb = w * HALF
                for d in range(NCHW):
                    dsl = slice(wb + d * CW, wb + (d + 1) * CW)
                    db = wb + d * CW
                    pP = ps.tile([128, CW], f32, name="pP", tag="ps")
                    mm(pP, wl0p[m][:], pv, slice(0, 2), db)
                    nc.vector.tensor_scalar_max(hPa[:, dsl], pP[:], 0.0)
                    pQ = ps.tile([128, CW], f32, name="pQ", tag="ps")
                    mm(pQ[0:QW], wl0q[:, ts(i - 1, QW)], pv, slice(0, 67), db)
                    nc.scalar.activation(hQa[0:QW, dsl], pQ[0:QW], AF.Relu)
                    pP = ps.tile([128, CW], f32, name="pP", tag="ps")
                    mm(pP, whp[m][0][:], hPa, slice(0, 128), db)
                    nc.scalar.activation(hPb[:, dsl], pP[:], AF.Relu,
                                         bias=bhp[:, 3 * m + 0:3 * m + 1])
                    pQ = ps.tile([128, CW], f32, name="pQ", tag="ps")
                    mm(pQ[0:QW], whq[m][0][:], hQa, slice(0, 115), db)
                    nc.vector.tensor_scalar_max(hQb[0:QW, dsl], pQ[0:QW], 0.0)
                    pP = ps.tile([128, CW], f32, name="pP", tag="ps")
                    mm(pP, whp[m][1][:], hPb, slice(0, 128), db)
                    nc.scalar.activation(hPa[:, dsl], pP[:], AF.Relu,
                                         bias=bhp[:, 3 * m + 1:3 * m + 2])
                    pQ = ps.tile([128, CW], f32, name="pQ", tag="ps")
                    mm(pQ[0:QW], whq[m][1][:], hQb, slice(0, 115), db)
                    nc.scalar.activation(hQa[0:QW, dsl], pQ[0:QW], AF.Relu)
                    pP = ps.tile([128, CW], f32, name="pP", tag="ps")
                    mm(pP, whp[m][2][:], hPa, slice(0, 128), db)
                    nc.scalar.activation(hPb[:, dsl], pP[:], AF.Relu,
                                         bias=bhp[:, 3 * m + 2:3 * m + 3])
                    pQ = ps.tile([128, CW], f32, name="pQ", tag="ps")
                    mm(pQ[0:WD], wh3q[m][:], hQa, slice(0, 115), db)
                    nc.vector.tensor_scalar_max(hQb[0:WD, dsl], pQ[0:WD], 0.0)
                    pO = ps.tile([128, CW], f32, name="pO", tag="ps")
                    mm2(pO[0:3], wod3[m][:], hQb, slice(0, WD),
                        wop3[m][:], hPb, slice(0, 128), db)
                    nc.scalar.activation(bridge[:, dsl], pO[0:3], AF.Copy)

            def emit_B(i, m, w, disc, zs):
                wb = w * HALF
                bp = slice(w * PW, (w + 1) * PW)
                wsl = slice(wb, wb + HALF)
                g = nc.vector
                for (row, dst) in ((0, "diffr"), (1, "dror"), (2, "dvr")):
                    nc.sync.dma_start(
                        bt[dst][bp, :],
                        bridge[row:row + 1, wsl].rearrange("o (p f) -> o p f", p=PW),
                    )
                c_bd = consts[bp, 3 * m + 0:3 * m + 1]
                c_bv = consts[bp, 3 * m + 1:3 * m + 2]
                c_brh = consts[bp, 3 * m + 2:3 * m + 3]
                S, V = bt["S"][bp, :], bt["V"][bp, :]

                def softplus(dst, src, biasap):
                    xb = bt["xb"][bp, :]
                    ab = bt["ab"][bp, :]
                    nc.vector.tensor_scalar_add(xb, src, biasap)
                    nc.scalar.activation(ab, xb, AF.Abs)
                    nc.scalar.activation(ab, ab, AF.Exp, bias=0.0, scale=-1.0)
                    nc.scalar.activation(ab, ab, AF.Ln, bias=1.0, scale=1.0)
                    nc.vector.tensor_scalar_max(dst, xb, 0.0)
                    g.tensor_add(dst, dst, ab)

                softplus(bt["dsp"][bp, :], bt["diffr"][bp, :], c_bd)
                softplus(bt["vsp"][bp, :], bt["dvr"][bp, :], c_bv)

                t1, t2, t3 = bt["t1"][bp, :], bt["t2"][bp, :], bt["t3"][bp, :]
                rc1, rc2 = bt["r1"][bp, :], bt["r2"][bp, :]
                Sd, SdW, coef = bt["Sd"][bp, :], bt["SdW"][bp, :], bt["coef"][bp, :]
                # S chain (gpsimd, with DVE reciprocals for the two divides)
                nc.vector.tensor_scalar(t1, S, float(RATE * SQH), 1.0, OP.mult, OP.add)
                nc.vector.reciprocal(rc1, t1)
                nc.vector.tensor_scalar_mul(t2, S, float(RATE * HSTEP))
                g.tensor_mul(t1, t2, rc1)
                g.tensor_mul(Sd, S, bt["dsp"][bp, :])
                nc.vector.tensor_scalar(t2, Sd, float(SQH), 1.0, OP.mult, OP.add)
                nc.vector.reciprocal(rc2, t2)
                g.tensor_mul(SdW, Sd, dwb[bp, zs])
                g.tensor_mul(t2, SdW, rc2)
                nc.vector.tensor_scalar_mul(coef, SdW, disc)
                g.tensor_add(S, S, t1)
                g.tensor_add(S, S, t2)
                # V chain
                nc.vector.tensor_scalar_mul(t3, bt["dror"][bp, :], float(HSTEP))
                g.tensor_add(t3, t3, V)
                g.tensor_mul(t2, bt["vsp"][bp, :], dbb[bp, zs])
                g.tensor_add(t3, t3, t2)
                nc.vector.tensor_scalar(V, t3, c_brh, 0.0, OP.add, OP.max)
                g.tensor_tensor(bt["rmax"][bp, :], bt["rmax"][bp, :], S, OP.max)
                g.tensor_copy(varB[bp, ts(i, F)], V)

                nc.sync.dma_start(
                    pv[i + 2:i + 3, wsl].rearrange("o (p f) -> o p f", p=PW), r(S))
                nc.sync.dma_start(
                    pv[0:1, wsl].rearrange("o (p f) -> o p f", p=PW), r(V))
                nc.sync.dma_start(
                    sc[64:65, wsl].rearrange("o (p f) -> o p f", p=PW), r(coef))

            def emit_tail(i, m, w):
                wb = w * HALF
                for d in range(NCHW):
                    dsl = slice(wb + d * CW, wb + (d + 1) * CW)
                    db = wb + d * CW
                    pC = ps.tile([128, CW], f32, name="pC", tag="ps")
                    mm(pC[0:50], ones50[64:65, :], sc, slice(64, 65), db)
                    nc.vector.tensor_tensor(sc[0:50, dsl], hQa[WD:QW, dsl],
                                            pC[0:50], OP.mult)
                    pCV = ps.tile([128, CW], f32, name="pCV", tag="ps")
                    mm(pCV[0:27], woqc[m][:], sc, slice(0, 65), db)
                    nc.vector.tensor_add(cv[:, dsl], cv[:, dsl], pCV[0:27])

            for i in range(1, nsteps + 1):
              m = (i - 1) // PER
              disc = float(np.exp(-0.05 * tgrid[i - 1]))
              zs = ts(i - 1, F)
              emit_A(i, m, 0)
              emit_B(i, m, 0, disc, zs)
              emit_A(i, m, 1)
              emit_tail(i, m, 0)
              emit_B(i, m, 1, disc, zs)
              emit_tail(i, m, 1)
              if i == PER:
                  nc.sync.dma_start(o_cv32, cv[0:26, :])

            # --- final outputs ---
            nc.sync.dma_start(o_pv, pv[:].bitcast(f32))
            nc.sync.dma_start(o_var, varB[:])
            nc.sync.dma_start(o_cvf, cv[:])
            nc.sync.dma_start(o_rmax, bt["rmax"][:])

    nc.compile()
    return nc


def _get_program():
    if "nc" not in _CACHE:
        _CACHE["nc"] = _build_program()
    return _CACHE["nc"]


def kernel(**inputs):
    from concourse.bass_utils import run_bass_kernel_spmd

    sh, per_core = _host_prep(inputs)
    nc = _get_program()

    in_maps = []
    for c in range(NCORE):
        mp = dict(sh)
        mp.update(per_core[c])
        in_maps.append(mp)

    res = run_bass_kernel_spmd(nc, in_maps, list(range(NCORE)))
    outs = res.results

    # ---- host post-processing / gather ----
    path = np.zeros((MC, T + 1), np.float32)
    var_path = np.zeros((MC, T + 1), np.float32)
    cv32 = np.zeros((MC, 26), np.float32)
    cvf = np.zeros((MC, 27), np.float32)
    rmax = np.zeros((MC,), np.float32)
    for c in range(NCORE):
        o = outs[c]
        sl = slice(c * MCC, (c + 1) * MCC)
        path[sl] = o["o_pv"][2:67].T
        var_path[sl] = (
            o["o_var"].reshape(P, T + 1, F).transpose(0, 2, 1).reshape(MCC, T + 1)
        )
        cv32[sl] = o["o_cv32"].T
        cvf[sl] = o["o_cvf"].T
        rmax[sl] = o["o_rmax"].reshape(MCC)

    cv_ex = cvf[:, 26:27]
    S_T = path[:, T:T + 1]

    tg = np.linspace(0.0, 1.0, T + 1)
    pmat = np.zeros((NM, NS), np.float32)
    vpmat = np.zeros((NM, NS), np.float32)
    for mi, step in enumerate((PER, T)):
        S_m = path[:, step:step + 1].astype(np.float64)
        cvm = (cv32 if mi == 0 else cvf[:, 0:26]).astype(np.float64)
        cv3 = cvm.reshape(MC, NM, NS)[:, mi, :]
        price = np.exp(-0.05 * tg[step]) * np.maximum(S_m - STRIKES[None, :], 0.0) - cv3
        pmat[mi] = price.mean(0)
        vpmat[mi] = price.var(0, ddof=1)

    exo = (rmax[:, None] - S_T).astype(np.float64)
    discT = np.exp(-0.05 * tg[T])
    dexo = discT * exo
    exo_p = (dexo - cv_ex).astype(np.float32)
    mean_p = np.float32(exo_p.astype(np.float64).mean())
    var_p = np.float32(exo_p.astype(np.float64).var(ddof=1))
    error = (dexo - dexo.mean() - cv_ex).astype(np.float32)

    return (path, var_path, pmat, vpmat, exo_p, mean_p, var_p, error)
